# revision 32
# baseline (speedup 1.0000x reference)
"""MultiHeadAttention Bass/Tile kernel for Trainium2, 8 NeuronCores.

Sharding: (batch, head-half) -> 8 cores, zero collectives.
  core c: batch b = c//2, head half hh = c%2 (8 heads, full 2048 queries).
  Every projection column/row is computed exactly once across the 8 cores
  (no K/V duplication); the two head-halves' out-projection partial sums
  are added on the host at gather time (y[b] = part0 + part1, bf16
  partials upcast to f32).

Everything runs in bf16. The softmax P-pipeline (exp+mask over 33.5M
elements/core) has to keep pace with the PE's ~277us of matmul rows, so
each (ks, q-half) region of the score matrix is statically assigned one
of three production paths, balanced across ACT/DVE/Pool:
  C: one fused DVE op (affine_then_add): i16 = sp*(lam*128/ln2) + C2'
     + Bmask, i.e. a bf16-Schraudolph exp with the mask folded in
     ADDITIVELY (Bmask = -14080 where masked -> bits land in [~400,4000]
     -> bf16 ~2^-100; unmasked bits stay i16-safe for |s|<=83, data max
     is 77). The i16 tile is the bf16 P strip by bitcast. 1 op/element.
  A: ACT exp (f32 PSUM -> bf16) + DVE tensor_tensor mask-mult (2x mode).
  G: ACT exp + Pool tensor_tensor mask-mult (Pool cannot access PSUM, so
     it only ever touches the SBUF-side mult).
The mask is host-blended per region: additive (-14080*m) for C regions,
multiplicative (1-m) for A/G, so one [S,S] bf16 tensor serves all paths.

PSUM (8 banks): 5-deep ring of [128,512] f32 score tiles (also used by
the projection blocks) + PV accumulator pvd (2 banks) + denominator pvn
(1 bank). The 5-slot ring is the load-bearing choice: each score tile is
held ~1.1us (fill + sem + P-op) and reused every ~1.6us, so the PE never
stalls on PSUM turnaround (a 2-slot wide-tile variant measured 49% PE
occupancy in the attention heads).

Per-head per-ks-strip: 4 score matmuls (64-contract; PE cost is rows
only), 4 quarter P-ops (the G half emitted first so Pool's slow mults
get a head start), and the 5-strip-deferred PV accumulation in
[q,64]+den orientation (P strip stationary: 64+1 rows per qt, not 512).
After the PV drain the accumulator is evacuated to SBUF bf16 in two big
ops (so the single PSUM accumulator frees fast for the next head) and
the per-qt normalization runs as DVE tensor_scalar in 4x mode; OAn
head-pairs are transposed to OAT by batched XBAR SBUF->SBUF DMA
transposes (3D out AP = per-128x128-block transpose) as soon as each
head pair finishes (no PE transposes, no PSUM staging); K1-3 projections
run as per-(sn,g) fillers spaced across head 0's strips with V
just-in-time; mask strips stream in per-ks during head 0; the final
y[q,e] partial leaves in bf16 (upcast + summed with the other
head-half's partial on the host).
"""

import os
import sys

for _p in ("/opt/trn_rl_repo", "/root/.axon_site/_ro/trn_rl_repo"):
    if os.path.isdir(_p) and _p not in sys.path:
        sys.path.insert(0, _p)

from contextlib import ExitStack

import numpy as np
import ml_dtypes

import concourse.tile as tile
from concourse import bacc, mybir
from concourse.bass_utils import run_bass_kernel_spmd

B, S, D = 4, 2048, 1024
H, HD = 16, 64
HL = 8          # local heads per core
EC = HL * HD    # local embedding width (512)
QL = S          # full query rows per core
NCORES = 8

F32 = mybir.dt.float32
BF16 = mybir.dt.bfloat16
I16 = mybir.dt.int16
BF = ml_dtypes.bfloat16

LAM = 0.125
# bf16 Schraudolph: bits = s*LAM*128/ln2 + (16256 - 486408/65536 + 0.5)
FE_S0 = LAM * 128.0 / float(np.log(2.0))
FE_S1 = 16256.0 - 486408.0 / 65536.0 + 0.5
MASK_ADD = -14080.0  # additive mask in bits-domain for C regions

# path per (ks, q-half) unit u = 2*ks + half (both q-quarters of a half
# share the path; the host-blended mask form is per half).  13 C / 8 A /
# 11 G, hand-spaced so no ks strip carries two G (Pool) halves — Pool's
# 1.1us mults otherwise pile up and stall the deferred PV consumer.
_COUNTS = {"C": 13, "A": 8, "G": 11}


def _make_modes():
    used = {k: 0 for k in _COUNTS}
    out = []
    for i in range(32):
        pick = max(
            (k for k in _COUNTS if used[k] < _COUNTS[k]),
            key=lambda k: _COUNTS[k] * (i + 1) / 32.0 - used[k],
        )
        used[pick] += 1
        out.append(pick)
    return out


MODES = _make_modes()
G_FIRST = True

_NC_CACHE = {}


def _build_kernel(tc, t_in, t_out):
    nc = tc.nc
    qa_d, ka_d, va_d, m_d = t_in["qT"], t_in["kT"], t_in["vT"], t_in["mT"]
    wq_d, wk_d, wv_d, wo_d = t_in["wqT"], t_in["wkT"], t_in["wvT"], t_in["woT"]
    y = t_out["y"]

    def dchunks(dram):  # [D or EC, cols] -> [128, n, cols]
        return dram[:, :].rearrange("(c p) q -> p c q", p=128)

    with ExitStack() as ctx:
        persist = ctx.enter_context(tc.tile_pool(name="persist", bufs=1))
        QT8 = persist.tile([128, 4, QL], BF16)    # [e%128, hpair, q]
        KT8 = persist.tile([128, 4, S], BF16)     # [e%128, hpair, s]
        V = persist.tile([128, 16, HL, HD], BF16)  # [s%128, ks, h, d]
        ONEc = persist.tile([128, 1], BF16)
        # [q%128, hpair, qt, (h%2)*64+d] — hp-major so each head pair's
        # 16 qt x 128 panel is one contiguous 2D slice for the XBAR
        # transpose DMA.
        OAn = persist.tile([128, 4, 16, 128], BF16)

        nc.vector.memset(ONEc, 1.0)

        MBp = ctx.enter_context(tc.tile_pool(name="mbp", bufs=1, side="right"))
        MB = MBp.tile([128, 16, QL], BF16)        # blended mask strips

        def mb_dma(ks):
            nc.sync.dma_start(
                out=MB[:, ks, :],
                in_=m_d[128 * ks : 128 * (ks + 1), :]
                .rearrange("(s p) q -> p s q", p=128),
            )

        # evacuation engine rotation (PSUM reads: ACT/DVE only — GPSIMD
        # cannot access PSUM).
        nev = [0]

        def evac(out_ap, in_ap):
            if nev[0] % 2 == 0:
                nc.scalar.activation(
                    out=out_ap, in_=in_ap,
                    func=mybir.ActivationFunctionType.Copy,
                )
            else:
                nc.vector.tensor_copy(out=out_ap, in_=in_ap)
            nev[0] += 1

        stV_ctx = ExitStack()
        stV = stV_ctx.enter_context(
            tc.tile_pool(name="stageV", bufs=1, side="right")
        )
        WQ = stV.tile([128, 8, EC], BF16, tag="wq")
        nc.sync.dma_start(out=WQ[:, :, 0:128], in_=dchunks(wq_d)[:, :, 0:128])
        WK = stV.tile([128, 8, EC], BF16, tag="wk")
        WV = stV.tile([128, 8, EC], BF16, tag="wv")
        nc.sync.dma_start(out=WV, in_=dchunks(wv_d))

        with ExitStack() as p1ctx:
            qk_ring = p1ctx.enter_context(tc.tile_pool(name="qkr", bufs=2))
            ebuf = p1ctx.enter_context(tc.tile_pool(name="eb", bufs=6))
            pvs_pool = p1ctx.enter_context(tc.tile_pool(name="pvs", bufs=2))
            va_ring = p1ctx.enter_context(tc.tile_pool(name="var", bufs=2))
            ppool = p1ctx.enter_context(tc.tile_pool(name="pp", bufs=6))
            dpool = p1ctx.enter_context(tc.tile_pool(name="dr", bufs=2))
            psS = p1ctx.enter_context(
                tc.tile_pool(name="psS", bufs=5, space="PSUM")
            )
            psV = p1ctx.enter_context(
                tc.tile_pool(name="psV", bufs=1, space="PSUM")
            )

            # --- projection emitters (narrow blocks on the psS ring) ---
            def qp_dma(qn):
                qs = qk_ring.tile([128, 8, 512], BF16, tag="a")
                nc.sync.dma_start(
                    out=qs, in_=dchunks(qa_d)[:, :, 512 * qn : 512 * (qn + 1)]
                )
                return qs

            def proj_block(dst, W, src, qn, g):
                pb = psS.tile([128, 512], F32, tag="s")
                for c in range(8):
                    nc.tensor.matmul(
                        pb,
                        W[:, c, 128 * g : 128 * (g + 1)],
                        src[:, c, :],
                        start=(c == 0), stop=(c == 7),
                    )
                evac(dst[:, g, 512 * qn : 512 * (qn + 1)], pb)

            def kp_dma(sn):
                ks_ = qk_ring.tile([128, 8, 512], BF16, tag="a")
                nc.sync.dma_start(
                    out=ks_, in_=dchunks(ka_d)[:, :, 512 * sn : 512 * (sn + 1)]
                )
                return ks_

            def vp_dma(st):
                vs = va_ring.tile([128, 8, 128], BF16, tag="v")
                nc.sync.dma_start(
                    out=vs, in_=dchunks(va_d)[:, :, 128 * st : 128 * (st + 1)]
                )
                return vs

            def vp_block(st, vs):  # V rows [128st, 128st+128)
                pvp = psS.tile([128, 512], F32, tag="s")
                for c in range(8):
                    nc.tensor.matmul(
                        pvp,
                        vs[:, c, :],
                        WV[:, c, :],
                        start=(c == 0), stop=(c == 7),
                    )
                evac(
                    V[:, st, :, :],
                    pvp[:, :].rearrange("p (h d) -> p h d", h=HL),
                )

            def emit_pv(pvd, pvn, h, ks, Pk):
                # PSUM start=True zeroes the whole 2KB bank (zero region), so
                # only the first matmul touching each bank may set it; the
                # other subtile regions accumulate onto pending-zero bytes.
                for qt in range(16):
                    lhsT = Pk[:, 128 * qt : 128 * (qt + 1)]
                    nc.tensor.matmul(
                        pvd[:, qt, :], lhsT, V[:, ks, h, :],
                        start=(ks == 0 and qt % 8 == 0), stop=(ks == 15),
                        skip_group_check=True,
                    )
                    nc.tensor.matmul(
                        pvn[:, qt : qt + 1], lhsT, ONEc,
                        start=(ks == 0 and qt == 0), stop=(ks == 15),
                        skip_group_check=True,
                    )

            def pop_quarter(ks, half, qn, sp, Pk):
                qsl = slice(1024 * half + 512 * qn,
                            1024 * half + 512 * (qn + 1))
                msl = MB[:, ks, qsl]
                mode = MODES[2 * ks + half]
                if mode == "C":
                    nc.vector.affine_then_add(
                        out=Pk[:, qsl].bitcast(I16),
                        in0=sp, in1=msl,
                        scale=FE_S0, bias=FE_S1,
                    )
                else:
                    eb = ebuf.tile([128, 512], BF16)
                    nc.scalar.activation(
                        out=eb, in_=sp,
                        func=mybir.ActivationFunctionType.Exp,
                        scale=LAM,
                    )
                    if mode == "A":
                        nc.vector.tensor_tensor(
                            out=Pk[:, qsl], in0=eb, in1=msl,
                            op=mybir.AluOpType.mult,
                        )
                    else:
                        nc.gpsimd.tensor_tensor(
                            out=Pk[:, qsl], in0=eb, in1=msl,
                            op=mybir.AluOpType.mult,
                        )

            def head(h, fillers, fillers_mid):
                g, poff = h // 2, 64 * (h % 2)
                pvd = psV.tile([128, 16, HD], F32, tag="pvd")
                pvn = psV.tile([128, 16], F32, tag="pvn")
                pq = []
                for ks in range(16):
                    for f in fillers.get(ks, ()):
                        f()
                    Pk = ppool.tile([128, QL], BF16)
                    halves = (
                        (1, 0) if G_FIRST and MODES[2 * ks + 1] == "G"
                        else (0, 1)
                    )
                    for hi, half in enumerate(halves):
                        if hi == 1:
                            for f in fillers_mid.get(ks, ()):
                                f()
                        for qn in range(2):
                            sp = psS.tile([128, 512], F32, tag="s")
                            nc.tensor.matmul(
                                sp,
                                KT8[poff : poff + 64, g,
                                    128 * ks : 128 * (ks + 1)],
                                QT8[poff : poff + 64, g,
                                    1024 * half + 512 * qn :
                                    1024 * half + 512 * (qn + 1)],
                                start=True, stop=True,
                            )
                            pop_quarter(ks, half, qn, sp, Pk)
                    pq.append((ks, Pk))
                    if len(pq) > 5:
                        k2, p2 = pq.pop(0)
                        emit_pv(pvd, pvn, h, k2, p2)
                for k2, p2 in pq:
                    emit_pv(pvd, pvn, h, k2, p2)

                denr = dpool.tile([128, 16], F32)
                nc.vector.reciprocal(out=denr, in_=pvn)
                # evacuate pvd to SBUF bf16 in two big ops so the single
                # PSUM accumulator frees fast for the next head, then do
                # the per-qt normalization as DVE tensor_scalar in 4x mode
                # (all-2-byte SBUF operands, per-partition f32 scalar).
                pvdS = pvs_pool.tile([128, 16, HD], BF16)
                nc.scalar.activation(
                    out=pvdS[:, 0:8, :], in_=pvd[:, 0:8, :],
                    func=mybir.ActivationFunctionType.Copy,
                )
                nc.vector.tensor_copy(out=pvdS[:, 8:16, :], in_=pvd[:, 8:16, :])
                hp, dof = h // 2, 64 * (h % 2)
                for qt in range(16):
                    nc.vector.tensor_scalar(
                        out=OAn[:, hp, qt, dof : dof + 64],
                        in0=pvdS[:, qt, :],
                        scalar1=denr[:, qt : qt + 1], scalar2=None,
                        op0=mybir.AluOpType.mult,
                    )

            def transpose_hp(hp):
                # XBAR SBUF->SBUF DMA transpose of the finished head pair:
                # each [128q x 128e] qt block of OAn[:, hp] transposes into
                # OAT[:, hp] (3D out AP => per-block transpose along qt).
                for qg in range(4):
                    nc.sync.dma_start_transpose(
                        out=OAT[:, hp, 512 * qg : 512 * (qg + 1)]
                        .rearrange("p (t r) -> p t r", t=4),
                        in_=OAn[:, hp, 4 * qg : 4 * (qg + 1), :]
                        .rearrange("p t r -> p (t r)"),
                    )

            # ---- emission schedule ----
            qs0 = qk_ring.tile([128, 8, 512], BF16, tag="a")
            nc.sync.dma_start(
                out=qs0[:, 0:4, :], in_=dchunks(qa_d)[:, 0:4, 0:512]
            )
            nc.sync.dma_start(
                out=qs0[:, 4:8, :], in_=dchunks(qa_d)[:, 4:8, 0:512]
            )
            nc.sync.dma_start(
                out=WQ[:, :, 128:EC], in_=dchunks(wq_d)[:, :, 128:EC]
            )
            nc.sync.dma_start(out=WK, in_=dchunks(wk_d))
            qs_list = [qs0, qp_dma(1)]
            for qn in range(4):
                if qn + 2 < 4:
                    qs_list.append(qp_dma(qn + 2))
                for g in range(4):
                    proj_block(QT8, WQ, qs_list[qn], qn, g)
            ks0 = kp_dma(0)
            for g in range(4):
                proj_block(KT8, WK, ks0, 0, g)
            mb_dma(0)
            mb_dma(1)

            # head-0 fillers: K strips 1-3 per-(sn,g), V just-in-time, mask
            # strips streamed per-ks.  vp at slot start, kp_g mid-slot so the
            # psS ring's evacuations hide behind the scores stream.
            f0 = {ks: [] for ks in range(16)}
            fm = {ks: [] for ks in range(16)}
            kd = {}
            vd = {}
            for j in (1, 2, 3):
                f0[4 * j - 4].append(
                    lambda j=j: kd.__setitem__(j, kp_dma(j)))
                for g in range(4):
                    fm[4 * j - 3 + g].append(
                        lambda j=j, g=g: proj_block(KT8, WK, kd[j], j, g))
            for ks in range(2, 16):
                f0[ks - 2].append(lambda ks=ks: mb_dma(ks))
            vd[0] = vp_dma(0)
            vd[1] = vp_dma(1)
            for st in range(16):
                if st + 2 < 16:
                    f0[st].append(
                        lambda st=st: vd.__setitem__(st + 2, vp_dma(st + 2)))
                f0[st].append(lambda st=st: vp_block(st, vd[st]))

            head(0, f0, fm)
            stV_ctx.close()
            p23 = ctx.enter_context(
                tc.tile_pool(name="p23", bufs=1, side="right")
            )
            OAT = p23.tile([128, 4, QL], BF16)  # OA^T [e%128, echunk, q]
            WO = p23.tile([128, 4, D], BF16, tag="wo")
            nc.sync.dma_start(out=WO, in_=dchunks(wo_d))

            for h in range(1, HL):
                head(h, {}, {})
                if h % 2 == 1:
                    transpose_hp(h // 2)

        # ---- tail: out projection, pipelined per q block; y in bf16 ----
        with (
            tc.tile_pool(name="yb", bufs=4) as ybuf,
            tc.tile_pool(name="psY", bufs=4, space="PSUM") as psY,
        ):
            for qt in range(16):
                yb = ybuf.tile([128, D], BF16)
                for en in range(2):
                    psy = psY.tile([128, 512], F32)
                    for c in range(4):
                        nc.tensor.matmul(
                            psy,
                            OAT[:, c, 128 * qt : 128 * (qt + 1)],
                            WO[:, c, 512 * en : 512 * (en + 1)],
                            start=(c == 0), stop=(c == 3),
                        )
                    evac(yb[:, 512 * en : 512 * (en + 1)], psy)
                nc.sync.dma_start(
                    out=y[128 * qt : 128 * (qt + 1), :], in_=yb
                )


def _get_nc():
    if "nc" in _NC_CACHE:
        return _NC_CACHE["nc"]
    nc = bacc.Bacc("TRN2", target_bir_lowering=False)
    t_in = {
        "qT": nc.dram_tensor("qT", [D, QL], BF16, kind="ExternalInput"),
        "kT": nc.dram_tensor("kT", [D, S], BF16, kind="ExternalInput"),
        "vT": nc.dram_tensor("vT", [D, S], BF16, kind="ExternalInput"),
        "mT": nc.dram_tensor("mT", [S, QL], BF16, kind="ExternalInput"),
        "wqT": nc.dram_tensor("wqT", [D, EC], BF16, kind="ExternalInput"),
        "wkT": nc.dram_tensor("wkT", [D, EC], BF16, kind="ExternalInput"),
        "wvT": nc.dram_tensor("wvT", [D, EC], BF16, kind="ExternalInput"),
        "woT": nc.dram_tensor("woT", [EC, D], BF16, kind="ExternalInput"),
    }
    t_out = {"y": nc.dram_tensor("y", [QL, D], BF16, kind="ExternalOutput")}
    with tile.TileContext(nc) as tc:
        _build_kernel(tc, t_in, t_out)
    nc.compile()
    _NC_CACHE["nc"] = nc
    return nc


def _bf(x):
    return np.asarray(x, np.float32).astype(BF)


def _blend_mask(mask_b):
    """[S, S] int mask -> blended bf16 [S(k), QL(q)]: additive bits-domain
    for C regions, multiplicative (1-m) for A/G regions."""
    mT = np.ascontiguousarray(mask_b.T).astype(np.float32)  # [k, q]
    out = np.empty_like(mT)
    for ks in range(16):
        for half in range(2):
            rs = slice(128 * ks, 128 * (ks + 1))
            cs = slice(1024 * half, 1024 * (half + 1))
            if MODES[2 * ks + half] == "C":
                out[rs, cs] = MASK_ADD * mT[rs, cs]
            else:
                out[rs, cs] = 1.0 - mT[rs, cs]
    return out.astype(BF)


def _in_maps(inputs):
    q = np.asarray(inputs["query"], np.float32)
    k = np.asarray(inputs["key"], np.float32)
    v = np.asarray(inputs["value"], np.float32)
    mask = np.asarray(inputs["mask"], np.int32)
    wqT = np.asarray(inputs["wq"], np.float32).T
    wkT = np.asarray(inputs["wk"], np.float32).T
    wvT = np.asarray(inputs["wv"], np.float32).T
    woT = np.asarray(inputs["w_out"], np.float32).T
    blends = [_blend_mask(mask[b]) for b in range(B)]
    maps = []
    for c in range(NCORES):
        b, hh = c // 2, c % 2
        esl = slice(hh * EC, (hh + 1) * EC)
        maps.append(
            {
                "qT": _bf(np.ascontiguousarray(q[b].T)),
                "kT": _bf(np.ascontiguousarray(k[b].T)),
                "vT": _bf(np.ascontiguousarray(v[b].T)),
                "mT": blends[b],
                "wqT": _bf(np.ascontiguousarray(wqT[:, esl])),
                "wkT": _bf(np.ascontiguousarray(wkT[:, esl])),
                "wvT": _bf(np.ascontiguousarray(wvT[:, esl])),
                "woT": _bf(np.ascontiguousarray(woT[esl, :])),
            }
        )
    return maps


def _gather(res):
    return np.stack(
        [
            res.results[2 * b]["y"].astype(np.float32)
            + res.results[2 * b + 1]["y"].astype(np.float32)
            for b in range(B)
        ]
    )


def kernel(**inputs) -> np.ndarray:
    nc = _get_nc()
    res = run_bass_kernel_spmd(nc, _in_maps(inputs), core_ids=list(range(NCORES)))
    return _gather(res)


def kernel_traced(**inputs):
    """Like kernel() but with NTFF tracing; returns (output, BassKernelResults)."""
    nc = _get_nc()
    res = run_bass_kernel_spmd(
        nc, _in_maps(inputs), core_ids=list(range(NCORES)), trace=True
    )
    return _gather(res), res


# revision 33
# speedup vs baseline: 1.0121x; 1.0121x over previous
"""MultiHeadAttention Bass/Tile kernel for Trainium2, 8 NeuronCores.

Sharding: (batch, head-half) -> 8 cores, zero collectives.
  core c: batch b = c//2, head half hh = c%2 (8 heads, full 2048 queries).
  Every projection column/row is computed exactly once across the 8 cores
  (no K/V duplication); the two head-halves' out-projection partial sums
  are added on the host at gather time (y[b] = part0 + part1, bf16
  partials upcast to f32).

Everything runs in bf16. The softmax P-pipeline (exp+mask over 33.5M
elements/core) has to keep pace with the PE's ~277us of matmul rows, so
each (ks, q-half) region of the score matrix is statically assigned one
of three production paths, balanced across ACT/DVE/Pool:
  C: one fused DVE op (affine_then_add): i16 = sp*(lam*128/ln2) + C2'
     + Bmask, i.e. a bf16-Schraudolph exp with the mask folded in
     ADDITIVELY (Bmask = -14080 where masked -> bits land in [~400,4000]
     -> bf16 ~2^-100; unmasked bits stay i16-safe for |s|<=83, data max
     is 77). The i16 tile is the bf16 P strip by bitcast. 1 op/element.
  A: ACT exp (f32 PSUM -> bf16) + DVE tensor_tensor mask-mult (2x mode).
  G: ACT exp + Pool tensor_tensor mask-mult (Pool cannot access PSUM, so
     it only ever touches the SBUF-side mult).
The mask is host-blended per region: additive (-14080*m) for C regions,
multiplicative (1-m) for A/G, so one [S,S] bf16 tensor serves all paths.

PSUM (8 banks): 5-deep ring of [128,512] f32 score tiles (also used by
the projection blocks) + PV accumulator pvd (2 banks) + denominator pvn
(1 bank). The 5-slot ring is the load-bearing choice: each score tile is
held ~1.1us (fill + sem + P-op) and reused every ~1.6us, so the PE never
stalls on PSUM turnaround (a 2-slot wide-tile variant measured 49% PE
occupancy in the attention heads).

Per-head per-ks-strip: 4 score matmuls (64-contract; PE cost is rows
only), 4 quarter P-ops (the G half emitted first so Pool's slow mults
get a head start), and the 5-strip-deferred PV accumulation in
[q,64]+den orientation (P strip stationary: 64+1 rows per qt, not 512).
After the PV drain the accumulator is evacuated to SBUF bf16 in two big
ops (so the single PSUM accumulator frees fast for the next head) and
the per-qt normalization runs as DVE tensor_scalar in 4x mode; OAn
head-pairs are transposed to OAT by batched XBAR SBUF->SBUF DMA
transposes (3D out AP = per-128x128-block transpose) as soon as each
head pair finishes (no PE transposes, no PSUM staging); K1-3 projections
run as per-(sn,g) fillers spaced across head 0's strips with V
just-in-time; mask strips stream in per-ks during head 0; the final
y[q,e] partial leaves in bf16 (upcast + summed with the other
head-half's partial on the host).
"""

import os
import sys

for _p in ("/opt/trn_rl_repo", "/root/.axon_site/_ro/trn_rl_repo"):
    if os.path.isdir(_p) and _p not in sys.path:
        sys.path.insert(0, _p)

from contextlib import ExitStack

import numpy as np
import ml_dtypes

import concourse.tile as tile
from concourse import bacc, mybir
from concourse.bass_utils import run_bass_kernel_spmd

B, S, D = 4, 2048, 1024
H, HD = 16, 64
HL = 8          # local heads per core
EC = HL * HD    # local embedding width (512)
QL = S          # full query rows per core
NCORES = 8

F32 = mybir.dt.float32
BF16 = mybir.dt.bfloat16
I16 = mybir.dt.int16
BF = ml_dtypes.bfloat16

LAM = 0.125
# bf16 Schraudolph: bits = s*LAM*128/ln2 + (16256 - 486408/65536 + 0.5)
FE_S0 = LAM * 128.0 / float(np.log(2.0))
FE_S1 = 16256.0 - 486408.0 / 65536.0 + 0.5
MASK_ADD = -14080.0  # additive mask in bits-domain for C regions

# path per (ks, q-half) unit u = 2*ks + half (both q-quarters of a half
# share the path; the host-blended mask form is per half).  13 C / 8 A /
# 11 G, hand-spaced so no ks strip carries two G (Pool) halves — Pool's
# 1.1us mults otherwise pile up and stall the deferred PV consumer.
_COUNTS = {"C": 13, "A": 8, "G": 11}


def _make_modes():
    used = {k: 0 for k in _COUNTS}
    out = []
    for i in range(32):
        pick = max(
            (k for k in _COUNTS if used[k] < _COUNTS[k]),
            key=lambda k: _COUNTS[k] * (i + 1) / 32.0 - used[k],
        )
        used[pick] += 1
        out.append(pick)
    return out


MODES = _make_modes()
G_FIRST = True

_NC_CACHE = {}


def _build_kernel(tc, t_in, t_out):
    nc = tc.nc
    qa_d, ka_d, va_d, m_d = t_in["qT"], t_in["kT"], t_in["vT"], t_in["mT"]
    wq_d, wk_d, wv_d, wo_d = t_in["wqT"], t_in["wkT"], t_in["wvT"], t_in["woT"]
    y = t_out["y"]

    def dchunks(dram):  # [D or EC, cols] -> [128, n, cols]
        return dram[:, :].rearrange("(c p) q -> p c q", p=128)

    with ExitStack() as ctx:
        persist = ctx.enter_context(tc.tile_pool(name="persist", bufs=1))
        QT8 = persist.tile([128, 4, QL], BF16)    # [e%128, hpair, q]
        KT8 = persist.tile([128, 4, S], BF16)     # [e%128, hpair, s]
        V = persist.tile([128, 16, HL, HD], BF16)  # [s%128, ks, h, d]
        ONEc = persist.tile([128, 1], BF16)
        # [q%128, hpair, qt, (h%2)*64+d] — hp-major so each head pair's
        # 16 qt x 128 panel is one contiguous 2D slice for the XBAR
        # transpose DMA.
        OAn = persist.tile([128, 4, 16, 128], BF16)

        nc.vector.memset(ONEc, 1.0)

        MBp = ctx.enter_context(tc.tile_pool(name="mbp", bufs=1, side="right"))
        MB = MBp.tile([128, 16, QL], BF16)        # blended mask strips

        def mb_dma(ks):
            nc.sync.dma_start(
                out=MB[:, ks, :],
                in_=m_d[128 * ks : 128 * (ks + 1), :]
                .rearrange("(s p) q -> p s q", p=128),
            )

        # evacuation engine rotation (PSUM reads: ACT/DVE only — GPSIMD
        # cannot access PSUM).
        nev = [0]

        def evac(out_ap, in_ap):
            if nev[0] % 2 == 0:
                nc.scalar.activation(
                    out=out_ap, in_=in_ap,
                    func=mybir.ActivationFunctionType.Copy,
                )
            else:
                nc.vector.tensor_copy(out=out_ap, in_=in_ap)
            nev[0] += 1

        stV_ctx = ExitStack()
        stV = stV_ctx.enter_context(
            tc.tile_pool(name="stageV", bufs=1, side="right")
        )
        WQ = stV.tile([128, 8, EC], BF16, tag="wq")
        nc.sync.dma_start(out=WQ[:, :, 0:128], in_=dchunks(wq_d)[:, :, 0:128])
        WK = stV.tile([128, 8, EC], BF16, tag="wk")
        WV = stV.tile([128, 8, EC], BF16, tag="wv")

        with ExitStack() as p1ctx:
            qk_ring = p1ctx.enter_context(tc.tile_pool(name="qkr", bufs=2))
            ebuf = p1ctx.enter_context(tc.tile_pool(name="eb", bufs=6))
            pvs_pool = p1ctx.enter_context(tc.tile_pool(name="pvs", bufs=2))
            va_ring = p1ctx.enter_context(tc.tile_pool(name="var", bufs=2))
            ppool = p1ctx.enter_context(tc.tile_pool(name="pp", bufs=6))
            dpool = p1ctx.enter_context(tc.tile_pool(name="dr", bufs=2))
            psS = p1ctx.enter_context(
                tc.tile_pool(name="psS", bufs=5, space="PSUM")
            )
            psV = p1ctx.enter_context(
                tc.tile_pool(name="psV", bufs=1, space="PSUM")
            )

            # --- projection emitters (narrow blocks on the psS ring) ---
            def qp_dma(qn):
                qs = qk_ring.tile([128, 8, 512], BF16, tag="a")
                nc.sync.dma_start(
                    out=qs, in_=dchunks(qa_d)[:, :, 512 * qn : 512 * (qn + 1)]
                )
                return qs

            def proj_block(dst, W, src, qn, g):
                pb = psS.tile([128, 512], F32, tag="s")
                for c in range(8):
                    nc.tensor.matmul(
                        pb,
                        W[:, c, 128 * g : 128 * (g + 1)],
                        src[:, c, :],
                        start=(c == 0), stop=(c == 7),
                    )
                evac(dst[:, g, 512 * qn : 512 * (qn + 1)], pb)

            def kp_dma(sn):
                ks_ = qk_ring.tile([128, 8, 512], BF16, tag="a")
                nc.sync.dma_start(
                    out=ks_, in_=dchunks(ka_d)[:, :, 512 * sn : 512 * (sn + 1)]
                )
                return ks_

            def vp_dma(st):
                vs = va_ring.tile([128, 8, 128], BF16, tag="v")
                nc.sync.dma_start(
                    out=vs, in_=dchunks(va_d)[:, :, 128 * st : 128 * (st + 1)]
                )
                return vs

            def vp_block(st, vs):  # V rows [128st, 128st+128)
                pvp = psS.tile([128, 512], F32, tag="s")
                for c in range(8):
                    nc.tensor.matmul(
                        pvp,
                        vs[:, c, :],
                        WV[:, c, :],
                        start=(c == 0), stop=(c == 7),
                    )
                evac(
                    V[:, st, :, :],
                    pvp[:, :].rearrange("p (h d) -> p h d", h=HL),
                )

            def emit_pv(pvd, pvn, h, ks, Pk):
                # PSUM start=True zeroes the whole 2KB bank (zero region), so
                # only the first matmul touching each bank may set it; the
                # other subtile regions accumulate onto pending-zero bytes.
                for qt in range(16):
                    lhsT = Pk[:, 128 * qt : 128 * (qt + 1)]
                    nc.tensor.matmul(
                        pvd[:, qt, :], lhsT, V[:, ks, h, :],
                        start=(ks == 0 and qt % 8 == 0), stop=(ks == 15),
                        skip_group_check=True,
                    )
                    nc.tensor.matmul(
                        pvn[:, qt : qt + 1], lhsT, ONEc,
                        start=(ks == 0 and qt == 0), stop=(ks == 15),
                        skip_group_check=True,
                    )

            def pop_quarter(ks, half, qn, sp, Pk):
                qsl = slice(1024 * half + 512 * qn,
                            1024 * half + 512 * (qn + 1))
                msl = MB[:, ks, qsl]
                mode = MODES[2 * ks + half]
                if mode == "C":
                    nc.vector.affine_then_add(
                        out=Pk[:, qsl].bitcast(I16),
                        in0=sp, in1=msl,
                        scale=FE_S0, bias=FE_S1,
                    )
                else:
                    eb = ebuf.tile([128, 512], BF16)
                    nc.scalar.activation(
                        out=eb, in_=sp,
                        func=mybir.ActivationFunctionType.Exp,
                        scale=LAM,
                    )
                    if mode == "A":
                        nc.vector.tensor_tensor(
                            out=Pk[:, qsl], in0=eb, in1=msl,
                            op=mybir.AluOpType.mult,
                        )
                    else:
                        nc.gpsimd.tensor_tensor(
                            out=Pk[:, qsl], in0=eb, in1=msl,
                            op=mybir.AluOpType.mult,
                        )

            def head(h, fillers, fillers_mid):
                g, poff = h // 2, 64 * (h % 2)
                pvd = psV.tile([128, 16, HD], F32, tag="pvd")
                pvn = psV.tile([128, 16], F32, tag="pvn")
                pq = []
                for ks in range(16):
                    for f in fillers.get(ks, ()):
                        f()
                    Pk = ppool.tile([128, QL], BF16)
                    halves = (
                        (1, 0) if G_FIRST and MODES[2 * ks + 1] == "G"
                        else (0, 1)
                    )
                    for hi, half in enumerate(halves):
                        if hi == 1:
                            for f in fillers_mid.get(ks, ()):
                                f()
                        for qn in range(2):
                            sp = psS.tile([128, 512], F32, tag="s")
                            nc.tensor.matmul(
                                sp,
                                KT8[poff : poff + 64, g,
                                    128 * ks : 128 * (ks + 1)],
                                QT8[poff : poff + 64, g,
                                    1024 * half + 512 * qn :
                                    1024 * half + 512 * (qn + 1)],
                                start=True, stop=True,
                            )
                            pop_quarter(ks, half, qn, sp, Pk)
                    pq.append((ks, Pk))
                    if len(pq) > 5:
                        k2, p2 = pq.pop(0)
                        emit_pv(pvd, pvn, h, k2, p2)
                for k2, p2 in pq:
                    emit_pv(pvd, pvn, h, k2, p2)

                denr = dpool.tile([128, 16], F32)
                nc.vector.reciprocal(out=denr, in_=pvn)
                # evacuate pvd to SBUF bf16 in two big ops so the single
                # PSUM accumulator frees fast for the next head, then do
                # the per-qt normalization as DVE tensor_scalar in 4x mode
                # (all-2-byte SBUF operands, per-partition f32 scalar).
                pvdS = pvs_pool.tile([128, 16, HD], BF16)
                nc.scalar.activation(
                    out=pvdS[:, 0:8, :], in_=pvd[:, 0:8, :],
                    func=mybir.ActivationFunctionType.Copy,
                )
                nc.vector.tensor_copy(out=pvdS[:, 8:16, :], in_=pvd[:, 8:16, :])
                hp, dof = h // 2, 64 * (h % 2)
                for qt in range(16):
                    nc.vector.tensor_scalar(
                        out=OAn[:, hp, qt, dof : dof + 64],
                        in0=pvdS[:, qt, :],
                        scalar1=denr[:, qt : qt + 1], scalar2=None,
                        op0=mybir.AluOpType.mult,
                    )

            def transpose_hp(hp):
                # XBAR SBUF->SBUF DMA transpose of the finished head pair:
                # each [128q x 128e] qt block of OAn[:, hp] transposes into
                # OAT[:, hp] (3D out AP => per-block transpose along qt).
                for qg in range(4):
                    nc.sync.dma_start_transpose(
                        out=OAT[:, hp, 512 * qg : 512 * (qg + 1)]
                        .rearrange("p (t r) -> p t r", t=4),
                        in_=OAn[:, hp, 4 * qg : 4 * (qg + 1), :]
                        .rearrange("p t r -> p (t r)"),
                    )

            # ---- emission schedule ----
            qs0 = qk_ring.tile([128, 8, 512], BF16, tag="a")
            nc.sync.dma_start(
                out=qs0[:, 0:4, :], in_=dchunks(qa_d)[:, 0:4, 0:512]
            )
            nc.sync.dma_start(
                out=qs0[:, 4:8, :], in_=dchunks(qa_d)[:, 4:8, 0:512]
            )
            nc.sync.dma_start(
                out=WQ[:, :, 128:EC], in_=dchunks(wq_d)[:, :, 128:EC]
            )
            nc.sync.dma_start(out=WK, in_=dchunks(wk_d))
            qs_list = [qs0, qp_dma(1)]
            for qn in range(4):
                if qn + 2 < 4:
                    qs_list.append(qp_dma(qn + 2))
                for g in range(4):
                    proj_block(QT8, WQ, qs_list[qn], qn, g)
            ks0 = kp_dma(0)
            nc.sync.dma_start(out=WV, in_=dchunks(wv_d))
            for g in range(4):
                proj_block(KT8, WK, ks0, 0, g)
            mb_dma(0)
            mb_dma(1)

            # head-0 fillers: K strips 1-3 per-(sn,g), V just-in-time, mask
            # strips streamed per-ks.  vp at slot start, kp_g mid-slot so the
            # psS ring's evacuations hide behind the scores stream.
            f0 = {ks: [] for ks in range(16)}
            fm = {ks: [] for ks in range(16)}
            kd = {}
            vd = {}
            for j in (1, 2, 3):
                f0[4 * j - 4].append(
                    lambda j=j: kd.__setitem__(j, kp_dma(j)))
                for g in range(4):
                    fm[4 * j - 3 + g].append(
                        lambda j=j, g=g: proj_block(KT8, WK, kd[j], j, g))
            for ks in range(2, 16):
                f0[ks - 2].append(lambda ks=ks: mb_dma(ks))
            vd[0] = vp_dma(0)
            vd[1] = vp_dma(1)
            for st in range(16):
                if st + 2 < 16:
                    f0[st].append(
                        lambda st=st: vd.__setitem__(st + 2, vp_dma(st + 2)))
                f0[st].append(lambda st=st: vp_block(st, vd[st]))

            head(0, f0, fm)
            stV_ctx.close()
            p23 = ctx.enter_context(
                tc.tile_pool(name="p23", bufs=1, side="right")
            )
            OAT = p23.tile([128, 4, QL], BF16)  # OA^T [e%128, echunk, q]
            WO = p23.tile([128, 4, D], BF16, tag="wo")
            nc.sync.dma_start(out=WO, in_=dchunks(wo_d))

            for h in range(1, HL):
                head(h, {}, {})
                if h % 2 == 1:
                    transpose_hp(h // 2)

        # ---- tail: out projection, pipelined per q block; y in bf16 ----
        with (
            tc.tile_pool(name="yb", bufs=4) as ybuf,
            tc.tile_pool(name="psY", bufs=4, space="PSUM") as psY,
        ):
            for qt in range(16):
                yb = ybuf.tile([128, D], BF16)
                for en in range(2):
                    psy = psY.tile([128, 512], F32)
                    for c in range(4):
                        nc.tensor.matmul(
                            psy,
                            OAT[:, c, 128 * qt : 128 * (qt + 1)],
                            WO[:, c, 512 * en : 512 * (en + 1)],
                            start=(c == 0), stop=(c == 3),
                        )
                    evac(yb[:, 512 * en : 512 * (en + 1)], psy)
                nc.sync.dma_start(
                    out=y[128 * qt : 128 * (qt + 1), :], in_=yb
                )


def _get_nc():
    if "nc" in _NC_CACHE:
        return _NC_CACHE["nc"]
    nc = bacc.Bacc("TRN2", target_bir_lowering=False)
    t_in = {
        "qT": nc.dram_tensor("qT", [D, QL], BF16, kind="ExternalInput"),
        "kT": nc.dram_tensor("kT", [D, S], BF16, kind="ExternalInput"),
        "vT": nc.dram_tensor("vT", [D, S], BF16, kind="ExternalInput"),
        "mT": nc.dram_tensor("mT", [S, QL], BF16, kind="ExternalInput"),
        "wqT": nc.dram_tensor("wqT", [D, EC], BF16, kind="ExternalInput"),
        "wkT": nc.dram_tensor("wkT", [D, EC], BF16, kind="ExternalInput"),
        "wvT": nc.dram_tensor("wvT", [D, EC], BF16, kind="ExternalInput"),
        "woT": nc.dram_tensor("woT", [EC, D], BF16, kind="ExternalInput"),
    }
    t_out = {"y": nc.dram_tensor("y", [QL, D], BF16, kind="ExternalOutput")}
    with tile.TileContext(nc) as tc:
        _build_kernel(tc, t_in, t_out)
    nc.compile()
    _NC_CACHE["nc"] = nc
    return nc


def _bf(x):
    return np.asarray(x, np.float32).astype(BF)


def _blend_mask(mask_b):
    """[S, S] int mask -> blended bf16 [S(k), QL(q)]: additive bits-domain
    for C regions, multiplicative (1-m) for A/G regions."""
    mT = np.ascontiguousarray(mask_b.T).astype(np.float32)  # [k, q]
    out = np.empty_like(mT)
    for ks in range(16):
        for half in range(2):
            rs = slice(128 * ks, 128 * (ks + 1))
            cs = slice(1024 * half, 1024 * (half + 1))
            if MODES[2 * ks + half] == "C":
                out[rs, cs] = MASK_ADD * mT[rs, cs]
            else:
                out[rs, cs] = 1.0 - mT[rs, cs]
    return out.astype(BF)


def _in_maps(inputs):
    q = np.asarray(inputs["query"], np.float32)
    k = np.asarray(inputs["key"], np.float32)
    v = np.asarray(inputs["value"], np.float32)
    mask = np.asarray(inputs["mask"], np.int32)
    wqT = np.asarray(inputs["wq"], np.float32).T
    wkT = np.asarray(inputs["wk"], np.float32).T
    wvT = np.asarray(inputs["wv"], np.float32).T
    woT = np.asarray(inputs["w_out"], np.float32).T
    blends = [_blend_mask(mask[b]) for b in range(B)]
    maps = []
    for c in range(NCORES):
        b, hh = c // 2, c % 2
        esl = slice(hh * EC, (hh + 1) * EC)
        maps.append(
            {
                "qT": _bf(np.ascontiguousarray(q[b].T)),
                "kT": _bf(np.ascontiguousarray(k[b].T)),
                "vT": _bf(np.ascontiguousarray(v[b].T)),
                "mT": blends[b],
                "wqT": _bf(np.ascontiguousarray(wqT[:, esl])),
                "wkT": _bf(np.ascontiguousarray(wkT[:, esl])),
                "wvT": _bf(np.ascontiguousarray(wvT[:, esl])),
                "woT": _bf(np.ascontiguousarray(woT[esl, :])),
            }
        )
    return maps


def _gather(res):
    return np.stack(
        [
            res.results[2 * b]["y"].astype(np.float32)
            + res.results[2 * b + 1]["y"].astype(np.float32)
            for b in range(B)
        ]
    )


def kernel(**inputs) -> np.ndarray:
    nc = _get_nc()
    res = run_bass_kernel_spmd(nc, _in_maps(inputs), core_ids=list(range(NCORES)))
    return _gather(res)


def kernel_traced(**inputs):
    """Like kernel() but with NTFF tracing; returns (output, BassKernelResults)."""
    nc = _get_nc()
    res = run_bass_kernel_spmd(
        nc, _in_maps(inputs), core_ids=list(range(NCORES)), trace=True
    )
    return _gather(res), res
